# revision 36
# baseline (speedup 1.0000x reference)
"""Trainium2 Bass kernel for nn_BaseLoftqLinear (4-bit quantized linear + LoRA + bias).

Computes: out = x @ dequant(W).T + (x @ A.T) @ B.T + bias
  x: [4, 2048, 4096] f32, W: [4096, 4096] 4-bit packed, A: [16, 4096], B: [4096, 16]

Strategy (column-parallel over out_features across 8 cores, transpose-free):
  - each core owns 512 out_features; x replicated
  - host pre-layouts everything so the device does NO PE transposes and the
    PE runs only the 2048 main matmuls:
      * x -> bf16, transposed to [IN_F, M] (k-major): loads directly as lhsT
      * quant indices unpacked to [IN_F, N] u8 (k-major layout)
      * compact scale table c1*weight_max [64, N] (broadcast-DMA'd per chunk)
      * lora fold: bad = (B@A).T + delta*scale  [IN_F, N] (weight prep)
  - device W-prep (per 128-k chunk): W_eff = idx * scale + bad, two DVE ops,
    result resident in SBUF as bf16 [128, 32*512]
  - GEMM: po[128m, 512n] += xT[128k,128m].T @ W_eff[128k, 512n] over 32
    k-chunks per m-tile; bias add on DVE; store f32
  - first 7 m-tiles consume W_eff chunks as they are produced, in geometric
    k-blocks (1,1,2,4,8,16) to amortize the ~215ns PSUM-bank-switch bubble;
    remaining 57 m-tiles run straight k-inner
  - host gathers the 8 [8192, 512] outputs -> [4, 2048, 4096]
"""
import os
import sys

for _p in ("/opt/trn_rl_repo", "/root/.axon_site/_ro/trn_rl_repo"):
    if os.path.isdir(_p) and _p not in sys.path:
        sys.path.insert(0, _p)
        break

import numpy as np
import ml_dtypes

import concourse.bass as bass
import concourse.bacc as bacc
import concourse.tile as tile
import concourse.mybir as mybir

dt = mybir.dt

# problem constants (hardcoded per spec)
B_, S_, IN_F, OUT_F, RANK = 4, 2048, 4096, 4096, 16
M = B_ * S_                    # 8192 tokens
N_CORES = 8
N = OUT_F // N_CORES           # 512 out_features per core
BLOCK = 64                     # quant block size (along in_features)
NBLK = IN_F // BLOCK           # 64 scale blocks along k
MT = M // 128                  # 64 m-tiles
KC = IN_F // 128               # 32 k-chunks
GA = 6                         # m-tiles overlapped with W-prep
KBLOCKS = [4, 4, 8, 16]        # k-run lengths for group A
SB = 8                         # m-tiles per x superblock
NSB = MT // SB                 # 8 superblocks
XGRP = 2                       # k-chunks per x DMA tile


def build_program(affine: bool, lut_vals):
    """Single-core Bass program (SPMD: same program on all 8 cores)."""
    nc = bacc.Bacc("TRN2", target_bir_lowering=False, debug=False,
                   num_devices=N_CORES)

    v_dt = dt.bfloat16 if affine else dt.float32
    xt = nc.dram_tensor("xt", [IN_F, M], dt.bfloat16, kind="ExternalInput")
    # packed per-k rows: [idx(512) | scale(512) | bad(512)]
    isb = nc.dram_tensor("isb", [IN_F, 3 * N], v_dt, kind="ExternalInput")
    bias = nc.dram_tensor("bias", [N], dt.float32, kind="ExternalInput")
    out = nc.dram_tensor("out", [M, N], dt.float32, kind="ExternalOutput")

    with tile.TileContext(nc) as tc:
        with (
            tc.tile_pool(name="const", bufs=1) as constp,
            tc.tile_pool(name="wt", bufs=1) as wtp,
            tc.tile_pool(name="ip", bufs=10) as ip_,
            tc.tile_pool(name="tp", bufs=6) as tp_,
            tc.tile_pool(name="xsb", bufs=32) as xp,
            tc.tile_pool(name="osb", bufs=4) as op_,
            tc.tile_pool(name="ps_out", bufs=8, space="PSUM") as ps_out,
        ):
            xt_t = xt[:, :].tensor

            def x_dma(g, q):
                """Load x tile (superblock g, chunk-group q): [128k, 4cc x 1024m]."""
                x4 = xp.tile([128, XGRP * SB * 128], dt.bfloat16, tag="x4")
                src = bass.AP(
                    xt_t, (q * XGRP * 128) * M + g * (SB * 128),
                    [[M, 128], [128 * M, XGRP], [1, SB * 128]],
                )
                nc.gpsimd.dma_start(out=x4[:], in_=src)
                return x4

            def x_slice(x4, c, j):
                """lhsT [128k, 128m] for k-chunk c, local m-tile j."""
                ap = x4[:]
                off = (c % XGRP) * (SB * 128) + j * 128
                return bass.AP(ap.tensor, ap.offset + off,
                               [list(ap.ap[0]), [1, 128]])

            bias_sb = constp.tile([128, N], dt.float32, name="bias_sb")
            xt_tiles = {}
            NQ = KC // XGRP  # 8 chunk-groups per superblock

            # W_eff resident: wt_sb[:, c*N + nn] = W_eff[c*128 + p, nn]
            wt_sb = wtp.tile([128, KC * N], dt.bfloat16, name="wt_sb")

            po_A = []
            for _j in range(GA):
                poa = ps_out.tile([128, N], dt.float32, tag="po")
                po_A.append(poa)

            def emit_groupA_block(c0, c1):
                """k-run [c0, c1) for each of the GA early m-tiles."""
                for j in range(GA):
                    for c in range(c0, c1):
                        nc.tensor.matmul(
                            po_A[j][:],
                            x_slice(xt_tiles[(0, c // XGRP)], c, j),
                            wt_sb[:, c * N:(c + 1) * N],
                            start=(c == 0), stop=(c == KC - 1),
                        )

            # ---- phase 1: W-prep interleaved with group-A k-blocks ----
            blk_end = []
            e = 0
            for b in KBLOCKS:
                e += b
                blk_end.append(e)

            for c in range(KC):
                # one wide DMA per chunk (3KB lines), alternating hwdge rings
                i_t = ip_.tile([128, 3 * N], v_dt, tag="i_t")
                half = 64
                nc.sync.dma_start(
                    out=i_t[0:half, :],
                    in_=isb[c * 128:c * 128 + half, :])
                nc.scalar.dma_start(
                    out=i_t[half:128, :],
                    in_=isb[c * 128 + half:(c + 1) * 128, :])
                iv, s_t, b_t = (i_t[:, 0:N], i_t[:, N:2 * N],
                                i_t[:, 2 * N:3 * N])

                if c == 0:
                    # deferred startup loads (emitted after chunk-0 loads so
                    # they don't delay them)
                    bsrc = bass.AP(bias[:].tensor, 0, [[0, 128], [1, N]])
                    nc.scalar.dma_start(out=bias_sb[:], in_=bsrc)
                    for q in range(4):
                        xt_tiles[(0, q)] = x_dma(0, q)

                if affine:
                    # W_eff = idx * scale + bad (two DVE ops)
                    t = tp_.tile([128, N], dt.bfloat16, tag="t")
                    nc.vector.tensor_tensor(t[:], iv, s_t,
                                            mybir.AluOpType.mult)
                else:
                    # general 16-entry codebook: idx -> sum_k lut[k]*(idx==k)
                    t = tp_.tile([128, N], dt.float32, tag="t")
                    nc.vector.memset(t[:], 0.0)
                    for k in range(16):
                        msk = tp_.tile([128, N], dt.float32, tag="msk")
                        nc.vector.tensor_scalar(msk[:], iv, float(k), None,
                                                mybir.AluOpType.is_equal)
                        nc.vector.tensor_scalar_mul(msk[:], msk[:],
                                                    float(lut_vals[k]))
                        nc.vector.tensor_tensor(t[:], t[:], msk[:],
                                                mybir.AluOpType.add)
                    nc.vector.tensor_tensor(t[:], t[:], s_t,
                                            mybir.AluOpType.mult)
                nc.vector.tensor_tensor(wt_sb[:, c * N:(c + 1) * N], t[:],
                                        b_t, mybir.AluOpType.add)

                # group-A runs chase the produced chunks in geometric blocks
                if c + 1 in blk_end:
                    bi = blk_end.index(c + 1)
                    c0 = blk_end[bi - 1] if bi > 0 else 0
                    emit_groupA_block(c0, c + 1)
                # stagger remaining x loads: sb0 groups 4..15 early, sb1 later
                if c <= 11:
                    xt_tiles[(0, c + 4)] = x_dma(0, c + 4)
                elif c >= 16:
                    q1 = c - 16
                    xt_tiles[(1, q1)] = x_dma(1, q1)

            for q in range(NQ):
                if (1, q) not in xt_tiles:
                    xt_tiles[(1, q)] = x_dma(1, q)

            def store(ms, po):
                o_sb = op_.tile([128, N], dt.float32, tag="o_sb")
                nc.vector.tensor_tensor(o_sb[:], po[:], bias_sb[:],
                                        mybir.AluOpType.add)
                nc.sync.dma_start(out=out[ms * 128:(ms + 1) * 128, :],
                                  in_=o_sb[:])

            for j in range(GA):
                store(j, po_A[j])

            # ---- tail: k-inner m-tiles with resident W_eff ----
            for ms in range(GA, MT):
                g, j = ms // SB, ms % SB
                # prefetch two tiles of superblock g+1 per m-tile
                if g + 1 < NSB:
                    for q in (2 * j, 2 * j + 1):
                        if (g + 1, q) not in xt_tiles:
                            xt_tiles[(g + 1, q)] = x_dma(g + 1, q)
                po = ps_out.tile([128, N], dt.float32, tag="po")
                for c in range(KC):
                    nc.tensor.matmul(
                        po[:],
                        x_slice(xt_tiles[(g, c // XGRP)], c, j),
                        wt_sb[:, c * N:(c + 1) * N],
                        start=(c == 0), stop=(c == KC - 1),
                    )
                store(ms, po)

    nc.compile()
    return nc


_cache = {}


def _affine_params(lut: np.ndarray):
    lut = np.asarray(lut, dtype=np.float32)
    c1 = float(lut[15] - lut[0]) / 15.0
    idx = np.arange(16, dtype=np.float32)
    affine = bool(
        np.max(np.abs(lut - (lut[0] + c1 * idx))) <= 1e-6 * max(1e-30, np.max(np.abs(lut)))
        and abs(c1) > 1e-20
    )
    delta = float(lut[0]) / c1 if affine else 0.0
    return affine, c1, delta


def _get_program(lut: np.ndarray):
    lut = np.asarray(lut, dtype=np.float32)
    affine, c1, delta = _affine_params(lut)
    key = (affine, round(c1, 12), round(delta, 12), tuple(np.round(lut, 10).tolist()))
    if key not in _cache:
        _cache[key] = build_program(affine, lut.tolist())
    return _cache[key]


def make_in_maps(inputs: dict):
    lut = np.asarray(inputs["lookup_table"], dtype=np.float32)
    affine, c1, delta = _affine_params(lut)
    v_np = ml_dtypes.bfloat16 if affine else np.float32

    x = np.asarray(inputs["x"], dtype=np.float32).reshape(M, IN_F)
    xt = np.ascontiguousarray(x.astype(ml_dtypes.bfloat16).T)  # [IN_F, M]

    pk_full = np.asarray(inputs["packed_qweight"]).astype(np.uint8).reshape(-1)
    idx_full = np.empty(pk_full.size * 2, np.uint8)
    idx_full[0::2] = pk_full & 15
    idx_full[1::2] = pk_full >> 4
    idx_full = idx_full.reshape(OUT_F, IN_F)

    wmax_full = np.asarray(inputs["weight_max"], dtype=np.float32).reshape(OUT_F, NBLK)
    lora_a = np.asarray(inputs["lora_A"], dtype=np.float32)
    lora_b = np.asarray(inputs["lora_B"], dtype=np.float32)
    bias_full = np.asarray(inputs["bias"], dtype=np.float32).reshape(-1)

    in_maps = []
    for i in range(N_CORES):
        o0, o1 = i * N, (i + 1) * N
        idxT = idx_full[o0:o1, :].T  # [IN_F, N] u8
        wm = wmax_full[o0:o1, :]  # [N, NBLK]
        scale = wm.T * (c1 if affine else 1.0)  # [NBLK, N]
        sful = np.repeat(scale, BLOCK, axis=0)  # [IN_F, N]
        # lora weight fold: (B@A).T shard + delta*scale (affine)
        bad = lora_a.T @ lora_b[o0:o1].T  # [IN_F, N]
        if affine:
            bad = bad + delta * sful
        isb = np.empty((IN_F, 3 * N), v_np)
        isb[:, 0:N] = idxT.astype(v_np)
        isb[:, N:2 * N] = sful.astype(v_np)
        isb[:, 2 * N:3 * N] = bad.astype(v_np)
        in_maps.append({
            "xt": xt,
            "isb": isb,
            "bias": bias_full[o0:o1],
        })
    return in_maps


def kernel(**inputs) -> np.ndarray:
    from concourse.bass_utils import run_bass_kernel_spmd

    nc = _get_program(inputs["lookup_table"])
    in_maps = make_in_maps(inputs)
    res = run_bass_kernel_spmd(nc, in_maps, core_ids=list(range(N_CORES)))
    outs = [np.asarray(r["out"], dtype=np.float32) for r in res.results]
    full = np.concatenate(outs, axis=1)  # [M, OUT_F]
    return full.reshape(B_, S_, OUT_F)
